# revision 10
# baseline (speedup 1.0000x reference)
"""GNN message-passing (3-layer GraphConv) on 8 Trainium2 NeuronCores.

Strategy: partition nodes across 8 cores by contiguous dst ranges. Per layer:
AllGather the feature table, dma_gather per-edge source rows (int16 indices,
4 source windows of 32768 rows), segment-sum via per-slab PE matmuls against
host-built 0/1 selection matrices into PSUM (transposed msg layout [C, 512]),
scale by 1/deg, apply self+neighbor weight matmuls, bias+activation on ACT,
transpose back to row layout, store to the next layer's table.

All compile-time structure (cell capacities, slab column windows) is computed
on the host from the union across the 8 cores, so one SPMD program serves all
cores; per-core variation lives entirely in the data (indices, SEL values).
"""
import math
import numpy as np

import concourse.bass as bass
import concourse.bacc as bacc
import concourse.mybir as mybir
import concourse.tile as tile
from concourse.bass_utils import run_bass_kernel_spmd
from concourse.masks import make_identity

P = 128
N = 131072
E = 2097152
NC = 8
NPC = N // NC          # nodes per core: 16384
SUPER = 512            # dsts per psum super-window
NSUP = NPC // SUPER    # 32 supers per core
NS = 4                 # src windows (int16 index reach)
SW = N // NS           # 32768 rows per src window
DPAD = 64              # table row padded to 64 f32 = 256B
CIN = (16, 32, 64)     # per-layer input widths
COUT = (32, 64, 128)   # per-layer output widths

LAST_HW_NS = -1        # cost-model estimate of on-device time, set by kernel()


def _prep(src, dst):
    """Host-side structure + per-core data. Returns (meta, per_core list)."""
    core = dst // NPC
    dloc = dst % NPC
    sup = dloc // SUPER
    swin = src // SW

    # counts per (core, sup, swin)
    key = (core.astype(np.int64) * NSUP + sup) * NS + swin
    cnt = np.bincount(key, minlength=NC * NSUP * NS).reshape(NC, NSUP, NS)
    cap = cnt.max(axis=0)                                   # [NSUP, NS]
    cell_slots = ((cap + 127) // 128) * 128                 # slab aligned
    cell_off = np.zeros((NSUP, NS), np.int64)               # slot offset of cell
    off = 0
    for g in range(NSUP):
        for s in range(NS):
            cell_off[g, s] = off
            off += cell_slots[g, s]
    tot_slots = int(off)
    n_slab = tot_slots // 128

    # per-core slot-ordered edge arrays
    order = np.lexsort((dloc, swin, sup, core))
    o_src, o_dloc, o_core = src[order], dloc[order], core[order]
    o_sup, o_swin = sup[order], swin[order]
    starts = np.zeros(NC * NSUP * NS + 1, np.int64)
    np.cumsum(np.bincount(
        (o_core.astype(np.int64) * NSUP + o_sup) * NS + o_swin,
        minlength=NC * NSUP * NS), out=starts[1:])

    idx16 = np.zeros((NC, tot_slots), np.int16)             # window-relative src
    drel = np.full((NC, tot_slots), -1, np.int32)           # dst col in super, -1 pad
    for c in range(NC):
        for g in range(NSUP):
            for s in range(NS):
                i = (c * NSUP + g) * NS + s
                n = int(starts[i + 1] - starts[i])
                o0 = int(cell_off[g, s])
                idx16[c, o0:o0 + n] = (o_src[starts[i]:starts[i + 1]] - s * SW).astype(np.int16)
                drel[c, o0:o0 + n] = o_dloc[starts[i]:starts[i + 1]] % SUPER

    # slab metadata: for slab k (slots 128k..128k+128) in cell (g, s):
    # union column range over cores; coverage-extended per super.
    slab_sup = np.zeros(n_slab, np.int32)
    slab_lo = np.full(n_slab, 10 ** 9, np.int64)
    slab_hi = np.full(n_slab, -1, np.int64)
    for g in range(NSUP):
        for s in range(NS):
            o0, nsl = int(cell_off[g, s]), int(cell_slots[g, s]) // 128
            for t in range(nsl):
                k = o0 // 128 + t
                slab_sup[k] = g
                seg = drel[:, o0 + t * 128: o0 + (t + 1) * 128]
                vals = seg[seg >= 0]
                if vals.size:
                    slab_lo[k] = min(slab_lo[k], int(vals.min()))
                    slab_hi[k] = max(slab_hi[k], int(vals.max()))
    # slabs with no edges anywhere: give them a dummy 1-col range
    empty = slab_hi < 0
    slab_lo[empty] = 0
    slab_hi[empty] = 0
    # coverage fix: every column of every super must be written by some slab
    for g in range(NSUP):
        ks = np.where(slab_sup == g)[0]
        covered = np.zeros(SUPER, bool)
        for k in ks:
            covered[slab_lo[k]:slab_hi[k] + 1] = True
        miss = np.where(~covered)[0]
        for m in miss:
            # extend nearest slab's range
            k = ks[np.argmin(np.minimum(np.abs(slab_lo[ks] - m), np.abs(slab_hi[ks] - m)))]
            slab_lo[k] = min(slab_lo[k], m)
            slab_hi[k] = max(slab_hi[k], m)
            covered[slab_lo[k]:slab_hi[k] + 1] = True
    slab_w = (slab_hi - slab_lo + 1).astype(np.int64)
    sel_off = np.zeros(n_slab + 1, np.int64)
    np.cumsum(slab_w, out=sel_off[1:])
    sel_cols = int(sel_off[-1])

    # SEL int8 [NC, 128, sel_cols]
    sel = np.zeros((NC, 128, sel_cols), np.int8)
    slot_ids = np.arange(tot_slots)
    slab_of_slot = slot_ids // 128
    part_of_slot = slot_ids % 128
    for c in range(NC):
        valid = drel[c] >= 0
        k = slab_of_slot[valid]
        col = sel_off[k] + (drel[c][valid] - slab_lo[k])
        sel[c, part_of_slot[valid], col] = 1

    meta = dict(cell_slots=cell_slots, cell_off=cell_off, tot_slots=tot_slots,
                n_slab=n_slab, slab_sup=slab_sup, slab_lo=slab_lo,
                slab_w=slab_w, sel_off=sel_off, sel_cols=sel_cols)
    return meta, idx16, sel


def _wrap_idx(idx16_core, meta):
    """Per-call wrapped [128, tot/16] int16 layout (16-partition wrap, x8 replicas).
    Calls are per (super, swin) = cells, concatenated in slot order."""
    cols = meta["tot_slots"] // 16
    out = np.empty((128, cols), np.int16)
    for g in range(NSUP):
        for s in range(NS):
            o0 = int(meta["cell_off"][g, s])
            n = int(meta["cell_slots"][g, s])
            blk = idx16_core[o0:o0 + n].reshape(n // 16, 16).T   # [16, n/16]
            out[:, o0 // 16:(o0 + n) // 16] = np.tile(blk, (8, 1))
    return out


def _build(meta):
    nc = bacc.Bacc("TRN2", target_bir_lowering=False, debug=False,
                   num_devices=NC, num_swdge_queues=4)
    f32, i8, i16 = mybir.dt.float32, mybir.dt.int8, mybir.dt.int16

    cell_slots, cell_off = meta["cell_slots"], meta["cell_off"]
    tot_slots, n_slab = meta["tot_slots"], meta["n_slab"]
    slab_sup, slab_lo = meta["slab_sup"], meta["slab_lo"]
    slab_w, sel_off, sel_cols = meta["slab_w"], meta["sel_off"], meta["sel_cols"]

    # kernel I/O
    x_own = nc.dram_tensor("x_own", [NPC, DPAD], f32, kind="ExternalInput")
    xT_own = nc.dram_tensor("xT_own", [16, NPC], f32, kind="ExternalInput")
    idx_in = nc.dram_tensor("idx_in", [128, tot_slots // 16], i16, kind="ExternalInput")
    sel_in = nc.dram_tensor("sel_in", [128, sel_cols], i8, kind="ExternalInput")
    invdeg_in = nc.dram_tensor("invdeg_in", [1, NPC], f32, kind="ExternalInput")
    wa_in = [nc.dram_tensor(f"wa{l}", [CIN[l], COUT[l]], f32, kind="ExternalInput") for l in range(3)]
    wb_in = [nc.dram_tensor(f"wb{l}", [CIN[l], COUT[l]], f32, kind="ExternalInput") for l in range(3)]
    b_in = [nc.dram_tensor(f"b{l}", [COUT[l], 1], f32, kind="ExternalInput") for l in range(3)]
    out = nc.dram_tensor("out", [NPC, 128], f32, kind="ExternalOutput")

    act_fn = [mybir.ActivationFunctionType.Relu,
              mybir.ActivationFunctionType.Relu,
              mybir.ActivationFunctionType.Sigmoid]

    with tile.TileContext(nc) as tc:
        with (
            tc.tile_pool(name="dram", bufs=1, space="DRAM") as dr,
            tc.tile_pool(name="const", bufs=1) as cst,
            tc.tile_pool(name="gbuf", bufs=2) as gb,
            tc.tile_pool(name="selbuf", bufs=2) as slb,
            tc.tile_pool(name="small", bufs=2) as sm,
            tc.tile_pool(name="psum", bufs=2, space="PSUM") as ps,
            tc.tile_pool(name="psum2", bufs=2, space="PSUM") as ps2,
        ):
            # DRAM staging
            t_own = [dr.tile([NPC, DPAD], f32, name=f"t_own{l}") for l in range(3)]
            t_full = [dr.tile([N, DPAD], f32, name=f"t_full{l}", addr_space="Shared")
                      for l in range(3)]
            hT_bounce = [dr.tile([CIN[l], NPC], f32, name=f"hT{l}") for l in range(3)]

            # constants
            ones_row = cst.tile([1, 128], f32)
            nc.vector.memset(ones_row[:], 1.0)
            ident = cst.tile([P, P], f32)
            make_identity(nc, ident[:])
            w_a = [cst.tile([CIN[l], COUT[l]], f32, name=f"wa_t{l}") for l in range(3)]
            w_b = [cst.tile([CIN[l], COUT[l]], f32, name=f"wb_t{l}") for l in range(3)]
            b_t = [cst.tile([COUT[l], 1], f32, name=f"b_t{l}") for l in range(3)]
            for l in range(3):
                nc.sync.dma_start(out=w_a[l][:], in_=wa_in[l][:])
                nc.sync.dma_start(out=w_b[l][:], in_=wb_in[l][:])
                nc.sync.dma_start(out=b_t[l][:], in_=b_in[l][:])
            invdeg_t = cst.tile([1, NPC], f32)
            nc.sync.dma_start(out=invdeg_t[:], in_=invdeg_in[:])

            # stage x into t_own[0] and hT_bounce[0] (= xT_own)
            nc.sync.dma_start(out=t_own[0][:], in_=x_own[:])
            nc.sync.dma_start(out=hT_bounce[0][:], in_=xT_own[:])

            for l in range(3):
                ci, co = CIN[l], COUT[l]
                # AllGather this layer's table
                nc.gpsimd.collective_compute(
                    "AllGather", mybir.AluOpType.bypass,
                    replica_groups=[list(range(NC))],
                    ins=[t_own[l][:]], outs=[t_full[l][:]],
                )
                tbl = t_full[l]

                for g in range(NSUP):
                    gslots = int(cell_slots[g].sum())
                    g0 = int(cell_off[g, 0])
                    # gather the super's 4 cells
                    gt = gb.tile([P, gslots // P, DPAD], f32, tag="g")
                    for s in range(NS):
                        o0 = int(cell_off[g, s]) - g0
                        nsl = int(cell_slots[g, s])
                        if nsl == 0:
                            continue
                        it = sm.tile([P, nsl // 16], i16, tag="idx")
                        nc.sync.dma_start(
                            out=it[:],
                            in_=idx_in[:, (g0 + o0) // 16:(g0 + o0 + nsl) // 16])
                        nc.gpsimd.dma_gather(
                            out_ap=gt[:, o0 // 128:(o0 + nsl) // 128, :],
                            in_ap=tbl[s * SW:(s + 1) * SW, :],
                            idxs_ap=it[:],
                            num_idxs=nsl, num_idxs_reg=nsl, elem_size=DPAD,
                            single_packet=False, queue_num=s,
                        )

                    # SEL for this super's slabs: int8 -> f32 via ACT copy
                    k0, k1 = g0 // 128, (g0 + gslots) // 128
                    c0, c1 = int(sel_off[k0]), int(sel_off[k1])
                    sel8 = sm.tile([P, c1 - c0], i8, tag="sel8")
                    nc.sync.dma_start(out=sel8[:], in_=sel_in[:, c0:c1])
                    self_f = slb.tile([P, c1 - c0], f32, tag="self")
                    nc.scalar.activation(self_f[:], sel8[:],
                                         mybir.ActivationFunctionType.Copy)

                    # segment-sum into psum [ci, 512]
                    msgp = ps.tile([ci, SUPER], f32, space="PSUM", tag="msg")
                    for k in range(k0, k1):
                        jl, w = int(slab_lo[k]), int(slab_w[k])
                        so = int(sel_off[k]) - c0
                        nc.tensor.matmul(
                            out=msgp[:, jl:jl + w],
                            lhsT=gt[:, k - k0, 0:ci],
                            rhs=self_f[:, so:so + w],
                            start=(k == k0), stop=(k == k1 - 1),
                        )
                    # inv-deg broadcast [128, 512] via K=1 matmul
                    invp = ps2.tile([P, SUPER], f32, space="PSUM", tag="inv")
                    nc.tensor.matmul(
                        out=invp[:],
                        lhsT=ones_row[:, 0:128],
                        rhs=invdeg_t[:, g * SUPER:(g + 1) * SUPER],
                        start=True, stop=True)
                    inv_s = slb.tile([ci, SUPER], f32, tag="invs")
                    nc.scalar.activation(inv_s[:], invp[0:ci, :],
                                         mybir.ActivationFunctionType.Copy)
                    msg_s = slb.tile([ci, SUPER], f32, tag="msgs")
                    nc.vector.tensor_tensor(out=msg_s[:], in0=msgp[:],
                                            in1=inv_s[:],
                                            op=mybir.AluOpType.mult)
                    # self rhs
                    hT_c = sm.tile([ci, SUPER], f32, tag="hT")
                    nc.sync.dma_start(
                        out=hT_c[:],
                        in_=hT_bounce[l][:, g * SUPER:(g + 1) * SUPER])
                    # transform
                    hp = ps.tile([co, SUPER], f32, space="PSUM", tag="h")
                    nc.tensor.matmul(out=hp[:], lhsT=w_b[l][:], rhs=msg_s[:],
                                     start=True, stop=False)
                    nc.tensor.matmul(out=hp[:], lhsT=w_a[l][:], rhs=hT_c[:],
                                     start=False, stop=True)
                    hT_n = slb.tile([co, SUPER], f32, tag="hTn")
                    nc.scalar.activation(hT_n[:], hp[:], act_fn[l], bias=b_t[l][:])

                    if l < 2:
                        nc.sync.dma_start(
                            out=hT_bounce[l + 1][:, g * SUPER:(g + 1) * SUPER],
                            in_=hT_n[:])
                    # transpose to rows [512, co]: PE transpose per 128-col chunk
                    rows = sm.tile([P, 4 * co], f32, tag="rows")
                    for q in range(4):
                        tp = ps2.tile([P, co], f32, space="PSUM", tag="tp")
                        nc.tensor.transpose(
                            out=tp[:, 0:co],
                            in_=hT_n[:, q * 128:(q + 1) * 128],
                            identity=ident[0:co, 0:co])
                        nc.scalar.activation(rows[:, q * co:(q + 1) * co], tp[:, 0:co],
                                             mybir.ActivationFunctionType.Copy)
                    dst_rows = out if l == 2 else t_own[l + 1]
                    nc.sync.dma_start(
                        out=dst_rows[g * SUPER:(g + 1) * SUPER, 0:co].rearrange(
                            "(q p) c -> p q c", p=P),
                        in_=rows[:].rearrange("p (q c) -> p q c", q=4))
    nc.compile()
    return nc


def kernel(x, edge_index, batch, W_lin, b_lin,
           Ws3, Wn3, b3, Ws2, Wn2, b2, Ws1, Wn1, b1):
    x = np.asarray(x, np.float32)
    src = np.asarray(edge_index[0], np.int64).astype(np.int32)
    dst = np.asarray(edge_index[1], np.int64).astype(np.int32)

    meta, idx16, sel = _prep(src, dst)

    deg = np.bincount(dst, minlength=N).astype(np.float32)
    invdeg = 1.0 / np.maximum(deg, 1.0)

    # fused layer-3 weights (f64 for fidelity)
    W_lin64 = np.asarray(W_lin, np.float64)
    b_lin64 = np.asarray(b_lin, np.float64)
    wa = [np.asarray(W_lin64 @ np.asarray(Ws3, np.float64), np.float32),
          np.asarray(Ws2, np.float32), np.asarray(Ws1, np.float32)]
    wb = [np.asarray(W_lin64 @ np.asarray(Wn3, np.float64), np.float32),
          np.asarray(Wn2, np.float32), np.asarray(Wn1, np.float32)]
    bb = [np.asarray(b_lin64 @ (np.asarray(Ws3, np.float64) + np.asarray(Wn3, np.float64))
                     + np.asarray(b3, np.float64), np.float32),
          np.asarray(b2, np.float32), np.asarray(b1, np.float32)]

    nc = _build(meta)

    global LAST_HW_NS
    try:
        from concourse.timeline_sim import TimelineSim
        LAST_HW_NS = int(TimelineSim(nc).simulate())
    except Exception:
        LAST_HW_NS = -1

    in_maps = []
    for c in range(NC):
        xc = x[c * NPC:(c + 1) * NPC]
        x_pad = np.zeros((NPC, DPAD), np.float32)
        x_pad[:, :16] = xc
        im = {
            "x_own": x_pad,
            "xT_own": np.ascontiguousarray(xc.T),
            "idx_in": _wrap_idx(idx16[c], meta),
            "sel_in": sel[c],
            "invdeg_in": invdeg[c * NPC:(c + 1) * NPC][None, :],
        }
        for l in range(3):
            im[f"wa{l}"] = wa[l]
            im[f"wb{l}"] = wb[l]
            im[f"b{l}"] = bb[l][:, None]
        in_maps.append(im)

    res = run_bass_kernel_spmd(nc, in_maps, core_ids=list(range(NC)))
    return np.concatenate([res.results[c]["out"] for c in range(NC)], axis=0)
